# revision 8
# baseline (speedup 1.0000x reference)
"""Trainium2 Bass kernel for the fused 3-modality attention + FFN + softmax model.

v3: pass-count-minimal design. HW probes show every matmul costs
N_moving*0.417ns + ~17ns regardless of dtype/perf-mode (LDWEIGHTS fully
hidden), so the kernel minimizes PE passes: fp8 DoubleRow for q/k/v (K=256
per pass), fp8-DR for the LayerNorm feature reductions and the score
head-reduction (stats tolerate fp8), and exact algebraic folds everywhere
else:
  - bk dropped (adds a j-independent constant to scores: softmax-invariant)
  - g1/beta1 folded into W1/b1 on host; device LN1 emits plain (x-mu)*rs
  - LN2 never materialized: logits = x2 @ (g2*Ww) on-chip; the per-token
    rs2 and -mu2*rs2 are shipped to host, which applies
    z = pz*rs2 + (-mu2*rs2)*(g2@Ww) + (bw + beta2@Ww) before softmax.
    This removes the stats->broadcast->apply critical path entirely.
  - fp16 activations (vs bf16) everywhere for extra mantissa at equal speed

Layout: pure data parallel over 8 NeuronCores (batch sharded). Activations
feature-major: [128 partitions, chunk*tokens], DIM=1024 in KC=8 chunks.
"""

import numpy as np
import ml_dtypes

import concourse.bacc as bacc
import concourse.bass as bass
import concourse.mybir as mybir
import concourse.tile as tile

B, DIM, H, FFN, HD = 16384, 1024, 16, 4096, 64
NCORES = 8
TPC = B // NCORES          # tokens per core
TB = 512                   # token block (matmul moving dim)
KC = DIM // 128            # 8 feature chunks
MC1 = FFN // 128           # 32 ffn chunks
EPS = 1e-5

SA = 16.0                  # activation fp8 scale (x16 units)
SW = 256.0                 # weight fp8 scale for Wk/Wv
SWQ = 2048.0               # weight fp8 scale for Wq' and Wgq
SQK = 4096.0               # folded k-dequant carried inside qb (= SA*SW)

F16 = mybir.dt.float16
F32 = mybir.dt.float32
FP8 = mybir.dt.float8e4
AF = mybir.ActivationFunctionType
ALU = mybir.AluOpType
DR = mybir.MatmulPerfMode.DoubleRow

PSB = 8  # single rotating psum tag: all tiles are one bank each


def _c3(t, nper):
    """[128, nchunk*nper] tile AP -> [128, nchunk, nper] view."""
    return t[:].rearrange("p (c n) -> p c n", n=nper)


def _c4(t, nper):
    """[128, nchunk*nper] tile AP -> [128, npair, 2, nper] DR-pair view."""
    return t[:].rearrange("p (c two n) -> p c two n", two=2, n=nper)


def _emit(nc, tc, io, tpc):
    nblk = tpc // TB
    v, s, te, gp = nc.vector, nc.scalar, nc.tensor, nc.gpsimd

    with (
        tc.tile_pool(name="consts", bufs=1) as cp,
        tc.tile_pool(name="weights", bufs=1) as wp,
        tc.tile_pool(name="psum", bufs=1, space="PSUM") as pp,
        tc.tile_pool(name="work", bufs=1) as wk,
    ):
        # pin ACT table set 6 (Copy/Identity/Relu/Square/Exp/Ln) once
        nc.scalar.add_instruction(mybir.InstLoadActFuncSet(
            name=nc.get_next_instruction_name(), ins=[], outs=[],
            act_func_set_id=6))

        # ---- small constants ----
        C = {}
        for name, shape, dtype in (
            ("Ssel8", [128, 128], FP8), ("onec8", [128, 32], FP8),
            ("Eexp", [16, 1024], F16), ("one1", [1, 128], F16),
            ("qbias", [128, KC], F32),
            ("b1c", [128, MC1], F32), ("b2c", [128, KC], F32),
            ("g1c", [128, KC], F32),
            ("Wwt2", [128, 3 * KC], F16),
            ("epsc", [1, 1], F32), ("cv", [1, 1], F32), ("cm", [1, 1], F32),
        ):
            t = cp.tile(shape, dtype, name=f"c_{name}")
            nc.sync.dma_start(out=t[:], in_=io[name])
            C[name] = t

        # ---- resident fp8 weights (chunk-major: w[p, kc*N + n]) ----
        W = {}
        for name, width in (("Wqg", 2 * KC * DIM), ("Wk", KC * DIM),
                            ("Wv", KC * DIM)):
            t = wp.tile([128, width], FP8, name=f"w_{name}")
            W[name] = t
        nc.sync.dma_start(out=W["Wqg"][:], in_=io["Wqg"])
        wqg = _c3(W["Wqg"], DIM)
        wkc = _c3(W["Wk"], DIM)
        wvc = _c3(W["Wv"], DIM)

        def emit_inputs_dma(blk):
            """DMA this block's activations; q GEMM (2048-deep DR)."""
            t0 = blk * KC * TB
            st = {}
            qmv = wk.tile([128, 2 * KC * TB], FP8, tag="qmv", bufs=1,
                          name="qmv")
            nc.sync.dma_start(out=qmv[:],
                              in_=io["qmv"][:, 2 * t0:2 * t0 + 2 * KC * TB])
            mj = []
            for j in range(3):
                t = wk.tile([128, KC * TB], FP8, tag=f"m{j}", bufs=1,
                            name=f"m{j}")
                nc.sync.dma_start(out=t[:],
                                  in_=io[f"m{j}"][:, t0:t0 + KC * TB])
                mj.append(t)
            domb = wk.tile([128, KC * TB], F16, tag="domb", bufs=1,
                           name="domb")
            nc.sync.dma_start(out=domb[:],
                              in_=io["domb"][:, t0:t0 + KC * TB])
            st["qmv3"] = _c3(qmv, TB)
            st["mj3"] = [_c3(t, TB) for t in mj]
            st["domb"] = domb
            return st

        def emit_q(blk, st):
            qb = wk.tile([128, KC * TB], F16, tag="qb", bufs=1, name="qb")
            for mc in range(KC):
                pq = pp.tile([128, TB], F32, tag="ps", bufs=PSB, name="pq")
                for kp in range(KC):
                    te.matmul(pq[:],
                              wqg[:, 2 * kp:2 * kp + 2,
                                  mc * 128:(mc + 1) * 128],
                              st["qmv3"][:, 2 * kp:2 * kp + 2, :],
                              start=(kp == 0), stop=(kp == KC - 1),
                              perf_mode=DR)
                s.activation(qb[:, mc * TB:(mc + 1) * TB], pq[:],
                             AF.Identity, bias=C["qbias"][:, mc:mc + 1],
                             scale=1.0 / (SA * SWQ * SQK))
            st["qb"] = qb

        def emit_k_softmax(blk, st, inject=None):
            """Scores via fp8 tm pairs + DR head-select; 3-way softmax."""
            qb = st["qb"]
            scas = [pp.tile([16, TB], F32, tag="ps", bufs=PSB,
                            name=f"sca{j}") for j in range(3)]
            ssel4 = C["Ssel8"][:].rearrange("p (c two h) -> p c two h",
                                            two=2, h=16)
            pend = None
            for p2 in range(KC // 2):
                tms = [wk.tile([128, 2 * TB], FP8, tag=f"tm{j}", bufs=2,
                               name=f"tm{j}") for j in range(3)]
                for i in range(2):
                    mc = 2 * p2 + i
                    pks = []
                    for j in range(3):
                        pks.append(pp.tile([128, TB], F32, tag="ps",
                                           bufs=PSB, name=f"pk{j}"))
                    for kp in range(KC // 2):
                        for j in range(3):
                            te.matmul(pks[j][:],
                                      wkc[:, 2 * kp:2 * kp + 2,
                                          mc * 128:(mc + 1) * 128],
                                      st["mj3"][j][:, 2 * kp:2 * kp + 2, :],
                                      start=(kp == 0),
                                      stop=(kp == KC // 2 - 1),
                                      perf_mode=DR)
                    for j in range(3):
                        v.tensor_mul(tms[j][:, i * TB:(i + 1) * TB],
                                     pks[j][:], qb[:, mc * TB:(mc + 1) * TB])

                def sel(pp2, ptms):
                    for j in range(3):
                        te.matmul(scas[j][:],
                                  ssel4[:, pp2, :, :],
                                  ptms[j][:].rearrange(
                                      "p (two n) -> p two n", two=2),
                                  start=(pp2 == 0), stop=(pp2 == KC // 2 - 1),
                                  perf_mode=DR, skip_group_check=True)
                if pend is not None:
                    sel(*pend)
                pend = (p2, tms)
                if p2 == 1 and inject is not None:
                    inject()
            sel(*pend)

            ab = wk.tile([16, 3 * TB], F16, tag="ab", bufs=1, name="ab")
            for j in range(3):
                s.activation(ab[:, j * TB:(j + 1) * TB],
                             scas[j][:], AF.Exp)
            sm = wk.tile([16, TB], F16, tag="sm", bufs=1, name="sm")[:]
            lsm = wk.tile([16, TB], F16, tag="lsm", bufs=1, name="lsm")[:]
            rc = wk.tile([16, TB], F16, tag="rc", bufs=1, name="rc")[:]
            v.tensor_add(sm, ab[:, 0:TB], ab[:, TB:2 * TB])
            v.tensor_add(sm, sm, ab[:, 2 * TB:3 * TB])
            s.activation(lsm, sm, AF.Ln)
            s.activation(rc, lsm, AF.Exp, scale=-1.0)
            for j in range(3):
                abj = ab[:, j * TB:(j + 1) * TB]
                v.tensor_mul(abj, abj, rc)
            st["ab"] = ab

        def emit_v_xp(blk, st, inject=None):
            """Attention-weighted V + residual -> xp (fp16) + xp8/sq8 (fp8)."""
            ab = st["ab"]
            xp = wk.tile([128, KC * TB], F16, tag="xp", bufs=1, name="xp")
            xp8 = wk.tile([128, KC * TB], FP8, tag="xp8", bufs=1, name="xp8")
            sq8 = wk.tile([128, KC * TB], FP8, tag="sq8", bufs=1, name="sq8")
            for mc in range(KC):
                pvs = []
                for j in range(3):
                    pvs.append(pp.tile([128, TB], F32, tag="ps", bufs=PSB,
                                       name=f"pv{j}"))
                for kp in range(KC // 2):
                    for j in range(3):
                        te.matmul(pvs[j][:],
                                  wvc[:, 2 * kp:2 * kp + 2,
                                      mc * 128:(mc + 1) * 128],
                                  st["mj3"][j][:, 2 * kp:2 * kp + 2, :],
                                  start=(kp == 0), stop=(kp == KC // 2 - 1),
                                  perf_mode=DR)
                vts = []
                pas = []
                for j in range(3):
                    vt = wk.tile([128, TB], F16, tag="vt", bufs=3, name="vt")
                    s.activation(vt[:], pvs[j][:], AF.Copy)
                    vts.append(vt)
                    pa = pp.tile([128, TB], F32, tag="ps", bufs=PSB,
                                 name="pa")
                    te.matmul(pa[:], C["Eexp"][:, mc * 128:(mc + 1) * 128],
                              ab[:, j * TB:(j + 1) * TB],
                              start=True, stop=True)
                    pas.append(pa)
                acc = wk.tile([128, TB], F32, tag="acc", bufs=2, name="acc")
                t2 = wk.tile([128, TB], F32, tag="t2", bufs=1, name="t2")
                v.tensor_mul(acc[:], pas[0][:], vts[0][:])
                v.tensor_mul(t2[:], pas[1][:], vts[1][:])
                gp.tensor_add(acc[:], acc[:], t2[:])
                v.tensor_mul(t2[:], pas[2][:], vts[2][:])
                gp.tensor_add(acc[:], acc[:], t2[:])
                xpc = xp[:, mc * TB:(mc + 1) * TB]
                v.scalar_tensor_tensor(
                    xpc, acc[:], SA / SQK,
                    st["domb"][:, mc * TB:(mc + 1) * TB],
                    ALU.mult, ALU.add)
                s.activation(xp8[:, mc * TB:(mc + 1) * TB], xpc, AF.Copy)
                s.activation(sq8[:, mc * TB:(mc + 1) * TB], xpc, AF.Square,
                             scale=1.0 / SA)
                if mc == 1 and inject is not None:
                    inject()
            st["xp"] = xp
            st["xp8"] = xp8
            st["sq8"] = sq8

        def emit_ln_sums(blk, src8, sqs8, tag):
            """Feature-axis sum (rows 0:16) and sum-of-squares (rows 32:48)
            via fp8-DR ones-matmuls into one col-tiled psum bank."""
            pr_s = pp.tile([16, TB], F32, tag="ps", bufs=PSB,
                           name=f"prs{tag}")
            pr_q = pp.tile([16, TB], F32, tag="ps", bufs=PSB,
                           name=f"prq{tag}")
            oc = C["onec8"][:].rearrange("p (two h) -> p two h", two=2)
            s4 = _c4(src8, TB)
            q4 = _c4(sqs8, TB)
            for p2 in range(KC // 2):
                te.matmul(pr_s[:], oc, s4[:, p2, :, :],
                          start=(p2 == 0), stop=(p2 == KC // 2 - 1),
                          perf_mode=DR, skip_group_check=True)
            for p2 in range(KC // 2):
                te.matmul(pr_q[:], oc, q4[:, p2, :, :],
                          start=(p2 == 0), stop=(p2 == KC // 2 - 1),
                          perf_mode=DR, skip_group_check=True)
            return pr_s, pr_q

        def emit_ln_stats(blk, prt, tag, f32out=False):
            """4-op stats chain: rs = exp(-.5 ln(var+eps')), nmrs = -mu*rs."""
            dt = F32 if f32out else F16
            mu2 = wk.tile([1, TB], F32, tag=f"mu2{tag}", bufs=1, name="mu2")[:]
            var = wk.tile([1, TB], F32, tag=f"var{tag}", bufs=1, name="var")[:]
            lnv = wk.tile([1, TB], F32, tag=f"lnv{tag}", bufs=1, name="lnv")[:]
            rs = wk.tile([1, TB], dt, tag=f"rs{tag}", bufs=2, name="rs")[:]
            nmrs = wk.tile([1, TB], dt, tag=f"nm{tag}", bufs=2, name="nmrs")[:]
            pr_s, pr_q = prt
            s.activation(mu2, pr_s[0:1, :], AF.Square, scale=1.0 / DIM)
            v.scalar_tensor_tensor(var, pr_q[0:1, :], C["cv"][:], mu2,
                                   ALU.mult, ALU.subtract)
            s.activation(lnv, var, AF.Ln, bias=C["epsc"][:])
            s.activation(rs, lnv, AF.Exp, scale=-0.5)
            v.scalar_tensor_tensor(nmrs, pr_s[0:1, :], C["cm"][:], rs,
                                   ALU.mult, ALU.mult)
            return rs, nmrs

        def emit_ln1_bcast(blk, st):
            rs, nmrs = st["st1"]
            prs = pp.tile([128, TB], F32, tag="ps", bufs=PSB, name="prs")
            te.matmul(prs[:], C["one1"][:], rs, start=True, stop=True)
            pnm = pp.tile([128, TB], F32, tag="ps", bufs=PSB, name="pnm")
            te.matmul(pnm[:], C["one1"][:], nmrs, start=True, stop=True)
            rs_b = wk.tile([128, TB], F16, tag="rs_b", bufs=1, name="rs_b")
            s.activation(rs_b[:], prs[:], AF.Copy)
            nm_b = wk.tile([128, TB], F16, tag="nm_b", bufs=1, name="nm_b")
            s.activation(nm_b[:], pnm[:], AF.Copy)
            st["rs_b"], st["nm_b"] = rs_b, nm_b

        def emit_ln1_apply(blk, st):
            """xn = (xp - mu) * rs  (unit variance, g/beta folded into FFN)."""
            xp, rs_b, nm_b = st["xp"], st["rs_b"], st["nm_b"]
            xn = wk.tile([128, KC * TB], F16, tag="xn", bufs=1, name="xn")
            for kc in range(KC):
                eng = gp if kc % 2 == 0 else v
                xnc = xn[:, kc * TB:(kc + 1) * TB]
                eng.tensor_mul(xnc, xp[:, kc * TB:(kc + 1) * TB], rs_b[:])
                eng.tensor_add(xnc, xnc, nm_b[:])
            st["xn"] = xn

        def emit_ffn1(blk, st):
            """hb = relu(xn @ (16*g1*W1) + b1c), W1 streamed fp16."""
            xn = st["xn"]
            hb = wk.tile([128, MC1 * TB], F16, tag="hb", bufs=1, name="hb")
            for mc in range(MC1):
                w1t = wk.tile([128, KC * 128], F16, tag="w1s", bufs=2,
                              name="w1t")
                nc.sync.dma_start(
                    out=w1t[:],
                    in_=io["W1"][:, mc * KC * 128:(mc + 1) * KC * 128])
                ph = pp.tile([128, TB], F32, tag="ps", bufs=PSB, name="ph")
                for kc in range(KC):
                    te.matmul(ph[:], w1t[:, kc * 128:(kc + 1) * 128],
                              xn[:, kc * TB:(kc + 1) * TB],
                              start=(kc == 0), stop=(kc == KC - 1))
                dst = hb[:, mc * TB:(mc + 1) * TB]
                if mc % 2 == 0:
                    s.activation(dst, ph[:], AF.Relu,
                                 bias=C["b1c"][:, mc:mc + 1])
                else:
                    v.tensor_scalar(dst, ph[:], C["b1c"][:, mc:mc + 1], 0.0,
                                    ALU.add, ALU.max)
            st["hb"] = hb

        def emit_ffn2(blk, st):
            """x2 = xn*(16*g1) + (hb @ W2 + b2c); W2 streamed fp16."""
            xn, hb = st["xn"], st["hb"]
            x2 = wk.tile([128, KC * TB], F16, tag="x2", bufs=1, name="x2")
            x28 = wk.tile([128, KC * TB], FP8, tag="xp8", bufs=1, name="x28")
            s28 = wk.tile([128, KC * TB], FP8, tag="sq8", bufs=1, name="s28")
            HW2 = MC1 // 2
            for mc in range(KC):
                pf = pp.tile([128, TB], F32, tag="ps", bufs=PSB, name="pf")
                for half in range(2):
                    w2t = wk.tile([128, HW2 * 128], F16, tag="w2s", bufs=3,
                                  name="w2t")
                    base = mc * MC1 * 128 + half * HW2 * 128
                    nc.sync.dma_start(
                        out=w2t[:], in_=io["W2"][:, base:base + HW2 * 128])
                    for k2 in range(HW2):
                        kc = half * HW2 + k2
                        te.matmul(pf[:], w2t[:, k2 * 128:(k2 + 1) * 128],
                                  hb[:, kc * TB:(kc + 1) * TB],
                                  start=(kc == 0), stop=(kc == MC1 - 1))
                tf = wk.tile([128, TB], F16, tag="tf", bufs=2, name="tf")
                s.activation(tf[:], pf[:], AF.Identity,
                             bias=C["b2c"][:, mc:mc + 1])
                x2c = x2[:, mc * TB:(mc + 1) * TB]
                v.scalar_tensor_tensor(x2c, xn[:, mc * TB:(mc + 1) * TB],
                                       C["g1c"][:, mc:mc + 1], tf[:],
                                       ALU.mult, ALU.add)
                if mc % 2 == 0:
                    v.tensor_copy(x28[:, mc * TB:(mc + 1) * TB], x2c)
                else:
                    s.activation(x28[:, mc * TB:(mc + 1) * TB], x2c, AF.Copy)
                s.activation(s28[:, mc * TB:(mc + 1) * TB], x2c, AF.Square,
                             scale=1.0 / SA)
            st["x2"], st["x28"], st["s28"] = x2, x28, s28

        def emit_logits(blk, st):
            """pz = x2 @ (g2*Ww); rs2/nmrs2 shipped raw; host finishes LN2."""
            x2 = st["x2"]
            pz = pp.tile([3, TB], F32, tag="ps", bufs=PSB, name="pz")
            for kc in range(KC):
                te.matmul(pz[:], C["Wwt2"][:, kc * 3:(kc + 1) * 3],
                          x2[:, kc * TB:(kc + 1) * TB],
                          start=(kc == 0), stop=(kc == KC - 1),
                          skip_group_check=True)
            zt = wk.tile([3, TB], F32, tag="zt", bufs=1, name="zt")
            s.activation(zt[:], pz[:], AF.Copy)
            nc.sync.dma_start(out=io["zout"][0:3, blk * TB:(blk + 1) * TB],
                              in_=zt[:])
            rs2, nmrs2 = st["st2"]
            nc.sync.dma_start(out=io["zout"][3:4, blk * TB:(blk + 1) * TB],
                              in_=rs2)
            nc.sync.dma_start(out=io["zout"][4:5, blk * TB:(blk + 1) * TB],
                              in_=nmrs2)

        # ---- software-pipelined emission ----
        sts = [None] * nblk
        sts[0] = emit_inputs_dma(0)
        nc.sync.dma_start(out=W["Wk"][:], in_=io["Wk"])
        nc.sync.dma_start(out=W["Wv"][:], in_=io["Wv"])
        emit_q(0, sts[0])
        emit_k_softmax(0, sts[0])
        for blk in range(nblk):
            st = sts[blk]
            emit_v_xp(blk, st)
            st["pr1"] = emit_ln_sums(blk, st["xp8"], st["sq8"], "a")
            st["st1"] = emit_ln_stats(blk, st["pr1"], "a")
            if blk + 1 < nblk:
                sts[blk + 1] = emit_inputs_dma(blk + 1)
                emit_q(blk + 1, sts[blk + 1])
                emit_ln1_bcast(blk, st)
                emit_ln1_apply(blk, st)
                emit_k_softmax(blk + 1, sts[blk + 1])
            else:
                emit_ln1_bcast(blk, st)
                emit_ln1_apply(blk, st)
            emit_ffn1(blk, st)
            emit_ffn2(blk, st)
            st["pr2"] = emit_ln_sums(blk, st["x28"], st["s28"], "b")
            st["st2"] = emit_ln_stats(blk, st["pr2"], "b", f32out=True)
            emit_logits(blk, st)


def build_program(tpc=TPC):
    nc = bacc.Bacc("TRN2", target_bir_lowering=False, debug=False)
    io = {}

    def din(name, shape, dtype):
        io[name] = nc.dram_tensor(name, shape, dtype, kind="ExternalInput").ap()

    nblk = tpc // TB
    din("qmv", [128, nblk * 2 * KC * TB], FP8)
    for j in range(3):
        din(f"m{j}", [128, nblk * KC * TB], FP8)
    din("domb", [128, nblk * KC * TB], F16)
    din("Wqg", [128, 2 * KC * DIM], FP8)
    din("Wk", [128, KC * DIM], FP8)
    din("Wv", [128, KC * DIM], FP8)
    din("W1", [128, MC1 * KC * 128], F16)
    din("W2", [128, KC * MC1 * 128], F16)
    din("Ssel8", [128, 128], FP8)
    din("onec8", [128, 32], FP8)
    din("Eexp", [16, 1024], F16)
    din("one1", [1, 128], F16)
    din("qbias", [128, KC], F32)
    din("b1c", [128, MC1], F32)
    din("b2c", [128, KC], F32)
    din("g1c", [128, KC], F32)
    din("Wwt2", [128, 3 * KC], F16)
    din("epsc", [1, 1], F32)
    din("cv", [1, 1], F32)
    din("cm", [1, 1], F32)
    io["zout"] = nc.dram_tensor("zout", [5, tpc], F32,
                                kind="ExternalOutput").ap()

    with tile.TileContext(nc) as tc:
        _emit(nc, tc, io, tpc)
    nc.compile()
    return nc


def _chunk_cols(vec, width):
    """[width*128] host vector -> [128, width] chunk-column layout."""
    return np.ascontiguousarray(
        np.asarray(vec, np.float32).reshape(width, 128).T)


def _chunk_major(w, scale):
    """[Din, N] weight -> [128, (Din/128)*N] fp8 chunk-major, scaled."""
    f8 = ml_dtypes.float8_e4m3
    din, n = w.shape
    kc = din // 128
    out = (np.asarray(w, np.float32) * scale).reshape(kc, 128, n)
    out = out.transpose(1, 0, 2).reshape(128, kc * n)
    return np.ascontiguousarray(np.clip(out, -224, 224)).astype(f8)


def _stream_layout(w, nin, nout):
    """[Din, Dout] -> [128, nout*nin*128] fp16 per-out-chunk streaming
    tiles: t[p, mc*nin*128 + kc*128 + n] = w[kc*128+p, mc*128+n]."""
    out = np.asarray(w, np.float32).reshape(nin, 128, nout, 128)
    out = out.transpose(1, 2, 0, 3).reshape(128, nout * nin * 128)
    return np.ascontiguousarray(out).astype(np.float16)


def _act_blocks(x, tpc, scale, dtype):
    """[B, DIM] -> per-core list of [128, nblk*KC*TB] block-major tiles."""
    nblk = tpc // TB
    xs = (np.asarray(x, np.float32).T * scale)        # [DIM, B]
    out = []
    for c in range(xs.shape[1] // tpc):
        xc = xs[:, c * tpc:(c + 1) * tpc]             # [DIM, tpc]
        a = xc.reshape(KC, 128, nblk, TB).transpose(1, 2, 0, 3)
        a = np.ascontiguousarray(a.reshape(128, nblk * KC * TB))
        if dtype == "f8":
            a = np.clip(a, -224, 224).astype(ml_dtypes.float8_e4m3)
        else:
            a = a.astype(np.float16)
        out.append(a)
    return out


def prep_host_inputs(inputs, tpc=TPC, ncores=NCORES):
    f32 = np.float32
    f16 = np.float16
    f8 = ml_dtypes.float8_e4m3
    rt = 1.0 / np.sqrt(HD)

    Wq = np.asarray(inputs["Wq"], f32) * rt
    Wg = np.asarray(inputs["Wg"], f32)
    Wgq = (Wg @ Wq) / 3.0
    qbias = (np.asarray(inputs["bg"], f32) @ Wq
             + np.asarray(inputs["bq"], f32) * rt)

    g1 = np.asarray(inputs["g1"], f32)
    be1 = np.asarray(inputs["beta1"], f32)
    g2 = np.asarray(inputs["g2"], f32)
    be2 = np.asarray(inputs["beta2"], f32)
    W1 = np.asarray(inputs["W1"], f32)
    W2 = np.asarray(inputs["W2"], f32)
    Ww = np.asarray(inputs["Ww"], f32)
    b1 = np.asarray(inputs["b1"], f32)
    b2 = np.asarray(inputs["b2"], f32)
    bw = np.asarray(inputs["bw"], f32)

    # head-selector S[p, p2*32 + i*16 + h] (same linear layout as v2 Ssel)
    head_of = np.arange(DIM) // HD
    S = np.zeros((128, 128), f32)
    E = np.zeros((16, 1024), f32)
    for c in range(KC):
        for p in range(128):
            h = head_of[c * 128 + p]
            S[p, c * 16 + h] = 1.0
            E[h, c * 128 + p] = 1.0
    onec8 = np.zeros((128, 32), f32)
    onec8[:, 0] = 1.0
    onec8[:, 16] = 1.0

    consts = {
        "Wqg": np.concatenate([_chunk_major(Wq, SWQ),
                               _chunk_major(Wgq, SWQ)], axis=1),
        "Wk": _chunk_major(np.asarray(inputs["Wk"], f32), SW),
        "Wv": _chunk_major(np.asarray(inputs["Wv"], f32), SW),
        "W1": _stream_layout(SA * g1[:, None] * W1, KC, MC1),
        "W2": _stream_layout(W2, MC1, KC),
        "Ssel8": S.astype(f8),
        "onec8": onec8.astype(f8),
        "Eexp": E.astype(f16),
        "one1": np.ones((1, 128), f32).astype(f16),
        "qbias": _chunk_cols(qbias / SQK, KC),
        "b1c": _chunk_cols(SA * (b1 + be1 @ W1), MC1),
        "b2c": _chunk_cols(SA * (b2 + be1), KC),
        "g1c": _chunk_cols(SA * g1, KC),
        "Wwt2": np.ascontiguousarray(
            (g2[:, None] * Ww).reshape(KC, 128, 3)
            .transpose(1, 0, 2).reshape(128, 3 * KC)).astype(f16),
        "epsc": np.full((1, 1), SA * SA * EPS, f32),
        "cv": np.full((1, 1), SA * SA / DIM, f32),
        "cm": np.full((1, 1), -1.0 / DIM, f32),
    }

    m0 = np.asarray(inputs["m0"], f32)
    m1 = np.asarray(inputs["m1"], f32)
    m2 = np.asarray(inputs["m2"], f32)
    dom = np.asarray(inputs["domain_rep"], f32)
    msum = m0 + m1 + m2

    m0b = _act_blocks(m0, tpc, SA, "f8")
    m1b = _act_blocks(m1, tpc, SA, "f8")
    m2b = _act_blocks(m2, tpc, SA, "f8")
    domf = _act_blocks(dom, tpc, SA, "f8")
    msf = _act_blocks(msum, tpc, SA, "f8")
    domb = _act_blocks(dom + np.asarray(inputs["bv"], f32)[None, :],
                       tpc, SA, "f16")

    nblk = tpc // TB
    in_maps = []
    for c in range(ncores):
        m = dict(consts)
        q = np.empty((128, nblk * 2 * KC * TB), ml_dtypes.float8_e4m3)
        for b_ in range(nblk):
            q[:, b_ * 2 * KC * TB:b_ * 2 * KC * TB + KC * TB] = \
                domf[c][:, b_ * KC * TB:(b_ + 1) * KC * TB]
            q[:, b_ * 2 * KC * TB + KC * TB:(b_ + 1) * 2 * KC * TB] = \
                msf[c][:, b_ * KC * TB:(b_ + 1) * KC * TB]
        m["qmv"] = q
        m["m0"] = m0b[c]
        m["m1"] = m1b[c]
        m["m2"] = m2b[c]
        m["domb"] = domb[c]
        in_maps.append(m)

    zfix = {
        "sc": np.asarray(g2 @ Ww, np.float64),                 # [3]
        "bz": np.asarray(bw + be2 @ Ww, np.float64),           # [3]
    }
    return in_maps, zfix


def postprocess(results, zfix, ncores=NCORES):
    """results: list of per-core dicts with 'zout' [5, tpc]."""
    sc, bz = zfix["sc"], zfix["bz"]
    outs = []
    for c in range(ncores):
        r = np.asarray(results[c]["zout"], np.float64)         # [5, tpc]
        z = r[0:3] * r[3:4] + sc[:, None] * r[4:5] + bz[:, None]
        z = z.T                                                # [tpc, 3]
        z = z - z.max(axis=1, keepdims=True)
        e = np.exp(z)
        outs.append((e / e.sum(axis=1, keepdims=True)).astype(np.float32))
    return np.ascontiguousarray(np.concatenate(outs, axis=0))


def kernel(**inputs):
    from concourse.bass_utils import run_bass_kernel_spmd
    nc = build_program()
    in_maps, zfix = prep_host_inputs(inputs)
    res = run_bass_kernel_spmd(nc, in_maps, list(range(NCORES)))
    return postprocess([res.results[c] for c in range(NCORES)], zfix)
